# revision 13
# baseline (speedup 1.0000x reference)
"""Trainium2 Bass kernel for nn_AttentionLayerRouter.

Reference semantics: attention-pool over text_features, then a router MLP +
top-k — but the returned outputs depend ONLY on batch sample 0
(`top_indices[0], top_weights[0], layer_probs[0]`), so samples 1..15 are dead
computation and are skipped entirely.

Device work (the compute-heavy 99%): the pre-activation GEMM for sample 0,
preact = x @ W1.T with x = text_features[0] ([2048, 4096]), sharded across
the 8 cores along the CONTRACTION dim (512 of 4096 per core). Each core
computes a [256, 2048] partial in fp16-in/fp32-accumulate and returns it as
fp16. Sharding the contraction keeps per-core DMA at 2.3MB and lets every
matmul run with a 512-wide moving operand (PSUM-bank-sized) while rotating
across all 8 PSUM banks for back-to-back issue.

Host tail (~1% of FLOPs): sum the 8 partials, bias+relu, scores, softmax
over 2048, attention-weighted pooling, l2-normalize, router MLP on a single
[4096] vector, top-8 of 24. Validated against the reference: fp16 operands
+ fp16 partials perturb the final logits by ~4e-7 vs a 1.5e-5 minimum
top-k gap.
"""

import math

import numpy as np

import concourse.mybir as mybir
import concourse.tile as tile
from concourse import bacc
from concourse.bass_utils import run_bass_kernel_spmd

N_CORES = 8
SEQ = 2048
DIM = 4096
HID = 256
NUM_LAYERS = 24
TOP_ROUTER = 8
TEMPERATURE = 2.0

KSLICE = DIM // N_CORES  # 512 contraction elements per core
KT = KSLICE // 128  # 4 k-tiles per core
NQ = 4  # seq quarters (moving-operand N = 512 = one PSUM bank)
QW = SEQ // NQ  # 512
MH = HID // 128  # 2 hid tiles
CHUNK = HID + SEQ  # packed columns per k-tile: [w1t_k | xt_k]

_NC = None


def _build_nc():
    """Per-core program: partial[m*128+p, l] = sum_d W1T[d, .] x[l, d]."""
    nc = bacc.Bacc(
        "TRN2", target_bir_lowering=False, debug=False, num_devices=N_CORES
    )
    f16, f32 = mybir.dt.float16, mybir.dt.float32

    # One packed input: per k-tile, 256 cols of W1.T then 2048 cols of x.T
    # (both laid out partition-major on the host so every DMA is direct-2D).
    data = nc.dram_tensor("data", [128, KT * CHUNK], f16, kind="ExternalInput")
    ho_out = nc.dram_tensor("ho", [128, MH * SEQ], f16, kind="ExternalOutput")

    with tile.TileContext(nc) as tc:
        with (
            tc.tile_pool(name="sb", bufs=1) as sb,
            tc.tile_pool(name="psum", bufs=1, space="PSUM") as psum,
        ):
            chunks = []
            prev_dma = None
            for k in range(KT):
                ch = sb.tile([128, CHUNK], f16, tag=f"ch{k}", name=f"ch{k}")
                dma = nc.sync.dma_start(
                    out=ch[:, :], in_=data[:, k * CHUNK : (k + 1) * CHUNK]
                )
                if prev_dma is not None:
                    # Serialize transfers so chunk k lands before k+1 starts
                    # (parallel queues would split bandwidth and delay chunk 0,
                    # stalling the PE's cold-start).
                    tile.add_dep_helper(
                        dma.ins, prev_dma.ins, sync=True,
                        reason="serialize chunk DMAs",
                    )
                prev_dma = dma
                chunks.append(ch)

            hp = [
                psum.tile([128, SEQ], f32, tag=f"hp{m}", name=f"hp{m}")
                for m in range(MH)
            ]
            # k outer so matmuls track chunk arrivals; both hid-tiles
            # interleave per k (banks rotate across all 8 PSUM banks).
            ho = sb.tile([128, MH * SEQ], f16)
            for k in range(KT):
                ch = chunks[k]
                for m in range(MH):
                    w_blk = ch[:, m * 128 : (m + 1) * 128]
                    for q in range(NQ):
                        nc.tensor.matmul(
                            hp[m][:, q * QW : (q + 1) * QW],
                            w_blk,
                            ch[:, HID + q * QW : HID + (q + 1) * QW],
                            start=(k == 0),
                            stop=(k == KT - 1),
                        )
            # Evacuate PSUM as f16: DVE takes 6 banks, ACT the last 2
            # (DVE ~0.69us vs ACT ~1.4us per bank); outputs stream in
            # q-pair chunks so the last transfer is small.
            for m in range(MH):
                for qp in range(NQ // 2):
                    for q in (2 * qp, 2 * qp + 1):
                        dst = ho[:, m * SEQ + q * QW : m * SEQ + (q + 1) * QW]
                        srcp = hp[m][:, q * QW : (q + 1) * QW]
                        if m == 1 and q >= 2:
                            nc.scalar.activation(
                                out=dst,
                                in_=srcp,
                                func=mybir.ActivationFunctionType.Copy,
                                bias=0.0,
                                scale=1.0,
                            )
                        else:
                            nc.vector.tensor_copy(out=dst, in_=srcp)
                    nc.sync.dma_start(
                        out=ho_out[
                            :, m * SEQ + 2 * qp * QW : m * SEQ + 2 * (qp + 1) * QW
                        ],
                        in_=ho[
                            :, m * SEQ + 2 * qp * QW : m * SEQ + 2 * (qp + 1) * QW
                        ],
                    )

    nc.compile()
    return nc


def _device_partials(x, W1, trace=False):
    """Run the d-sharded partial GEMM; returns preact [256, 2048] f32 sum."""
    global _NC
    if _NC is None:
        _NC = _build_nc()

    xt16 = np.ascontiguousarray(x.T.astype(np.float16))  # [4096, 2048]
    wt16 = np.ascontiguousarray(W1.T.astype(np.float16))  # [4096, 256]

    in_maps = []
    for c in range(N_CORES):
        blk = np.empty((128, KT * CHUNK), dtype=np.float16)
        for k in range(KT):
            d0 = c * KSLICE + k * 128
            blk[:, k * CHUNK : k * CHUNK + HID] = wt16[d0 : d0 + 128]
            blk[:, k * CHUNK + HID : (k + 1) * CHUNK] = xt16[d0 : d0 + 128]
        in_maps.append({"data": blk})

    res = run_bass_kernel_spmd(
        _NC, in_maps, core_ids=list(range(N_CORES)), trace=trace
    )
    acc = np.zeros((HID, SEQ), dtype=np.float32)
    for c in range(N_CORES):
        ho = res.results[c]["ho"]  # [128, MH*SEQ] f16
        for m in range(MH):
            acc[m * 128 : (m + 1) * 128] += ho[:, m * SEQ : (m + 1) * SEQ]
    return acc, res


def _tail(s, x, b2, R1, Rb1, R2, Rb2):
    """Host fp32 tail: softmax -> pool -> normalize -> router MLP -> top-k."""
    s = (s + np.float32(b2.reshape(-1)[0])).astype(np.float32)
    m = s.max()
    e = np.exp(s - m)
    attn = (e / e.sum()).astype(np.float32)
    pooled = (attn @ x).astype(np.float32)  # [4096]
    nrm = np.float32(np.sqrt(np.float64((pooled.astype(np.float64) ** 2).sum())))
    pooled = pooled / max(nrm, np.float32(1e-12)) * np.float32(math.sqrt(DIM))
    pre = (pooled @ R1.T + Rb1).astype(np.float32)
    erf = np.array(
        [math.erf(float(v) / math.sqrt(2.0)) for v in pre], dtype=np.float32
    )
    r = np.float32(0.5) * pre * (np.float32(1.0) + erf)
    logits = (r @ R2.T + Rb2).astype(np.float32)
    lt = logits / np.float32(TEMPERATURE)
    em = np.exp(lt - lt.max())
    probs = (em / em.sum()).astype(np.float32)
    idx = np.argsort(-probs, kind="stable")[:TOP_ROUTER].astype(np.int32)
    w = probs[idx]
    w = (w / w.sum()).astype(np.float32)
    return idx, w, probs


def _run(inputs, trace=False):
    x = np.asarray(inputs["text_features"], dtype=np.float32)[0]
    W1 = np.asarray(inputs["W1"], dtype=np.float32)
    b1 = np.asarray(inputs["b1"], dtype=np.float32)
    W2 = np.asarray(inputs["W2"], dtype=np.float32)
    b2 = np.asarray(inputs["b2"], dtype=np.float32)
    R1 = np.asarray(inputs["R1"], dtype=np.float32)
    Rb1 = np.asarray(inputs["Rb1"], dtype=np.float32)
    R2 = np.asarray(inputs["R2"], dtype=np.float32)
    Rb2 = np.asarray(inputs["Rb2"], dtype=np.float32)

    preact, res = _device_partials(x, W1, trace=trace)
    h = np.maximum(preact + b1[:, None], 0.0).astype(np.float32)  # [256, 2048]
    s = (W2.reshape(-1).astype(np.float32) @ h).astype(np.float32)  # [2048]
    out = _tail(s, x, b2, R1, Rb1, R2, Rb2)
    return out, res


def kernel(**inputs):
    out, _ = _run(inputs, trace=False)
    return out


# revision 14
# speedup vs baseline: 1.2902x; 1.2902x over previous
"""Trainium2 Bass kernel for nn_AttentionLayerRouter.

Reference semantics: attention-pool over text_features, then a router MLP +
top-k — but the returned outputs depend ONLY on batch sample 0
(`top_indices[0], top_weights[0], layer_probs[0]`), so samples 1..15 are dead
computation and are skipped entirely.

Device work (the compute-heavy 99%): the pre-activation GEMM for sample 0,
preact = x @ W1.T with x = text_features[0] ([2048, 4096]), sharded across
the 8 cores along the CONTRACTION dim (512 of 4096 per core). Each core
computes a [256, 2048] partial in fp16-in/fp32-accumulate and returns it as
fp16. Sharding the contraction keeps per-core DMA at 2.3MB and lets every
matmul run with a 512-wide moving operand (PSUM-bank-sized) while rotating
across all 8 PSUM banks for back-to-back issue.

Host tail (~1% of FLOPs): sum the 8 partials, bias+relu, scores, softmax
over 2048, attention-weighted pooling, l2-normalize, router MLP on a single
[4096] vector, top-8 of 24. Validated against the reference: fp16 operands
+ fp16 partials perturb the final logits by ~4e-7 vs a 1.5e-5 minimum
top-k gap.
"""

import math

import numpy as np

import concourse.mybir as mybir
import concourse.tile as tile
from concourse import bacc
from concourse.bass_utils import run_bass_kernel_spmd

N_CORES = 8
SEQ = 2048
DIM = 4096
HID = 256
NUM_LAYERS = 24
TOP_ROUTER = 8
TEMPERATURE = 2.0

KSLICE = DIM // N_CORES  # 512 contraction elements per core
KT = KSLICE // 128  # 4 k-tiles per core
NQ = 4  # seq quarters (moving-operand N = 512 = one PSUM bank)
QW = SEQ // NQ  # 512
MH = HID // 128  # 2 hid tiles
CHUNK = HID + SEQ  # packed columns per k-tile: [w1t_k | xt_k]

_NC = None


def _build_nc():
    """Per-core program: partial[m*128+p, l] = sum_d W1T[d, .] x[l, d]."""
    nc = bacc.Bacc(
        "TRN2", target_bir_lowering=False, debug=False, num_devices=N_CORES
    )
    f16, f32 = mybir.dt.float16, mybir.dt.float32

    # One packed input: per k-tile, 256 cols of W1.T then 2048 cols of x.T
    # (both laid out partition-major on the host so every DMA is direct-2D).
    data = nc.dram_tensor("data", [128, KT * CHUNK], f16, kind="ExternalInput")
    ho_out = nc.dram_tensor("ho", [128, MH * SEQ], f16, kind="ExternalOutput")

    with tile.TileContext(nc) as tc:
        with (
            tc.tile_pool(name="sb", bufs=1) as sb,
            tc.tile_pool(name="psum", bufs=1, space="PSUM") as psum,
        ):
            # Warm the ACT lookup table early so the first real ACT copy
            # doesn't pay the ~1.3us table load.
            warm = sb.tile([128, 1], f32)
            nc.vector.memset(warm[:, :], 0.0)
            warm2 = sb.tile([128, 1], f16)
            nc.scalar.activation(
                out=warm2[:, :],
                in_=warm[:, :],
                func=mybir.ActivationFunctionType.Copy,
                bias=0.0,
                scale=1.0,
            )

            chunks = []
            for k in range(KT):
                ch = sb.tile([128, CHUNK], f16, tag=f"ch{k}", name=f"ch{k}")
                nc.sync.dma_start(
                    out=ch[:, :], in_=data[:, k * CHUNK : (k + 1) * CHUNK]
                )
                chunks.append(ch)

            hp = [
                psum.tile([128, SEQ], f32, tag=f"hp{m}", name=f"hp{m}")
                for m in range(MH)
            ]
            # k outer so matmuls track chunk arrivals; both hid-tiles
            # interleave per k (banks rotate across all 8 PSUM banks).
            ho = sb.tile([128, MH * SEQ], f16)
            for k in range(KT):
                ch = chunks[k]
                for m in range(MH):
                    w_blk = ch[:, m * 128 : (m + 1) * 128]
                    for q in range(NQ):
                        nc.tensor.matmul(
                            hp[m][:, q * QW : (q + 1) * QW],
                            w_blk,
                            ch[:, HID + q * QW : HID + (q + 1) * QW],
                            start=(k == 0),
                            stop=(k == KT - 1),
                        )
            # Evacuate PSUM as f16 with DVE and ACT in parallel (q0/q1 on
            # DVE, q2/q3 on ACT); outputs stream per (m, q-pair) so the last
            # transfer is small.
            for m in range(MH):
                for qp in range(NQ // 2):
                    for q in (2 * qp, 2 * qp + 1):
                        dst = ho[:, m * SEQ + q * QW : m * SEQ + (q + 1) * QW]
                        srcp = hp[m][:, q * QW : (q + 1) * QW]
                        if q >= 2:
                            nc.scalar.activation(
                                out=dst,
                                in_=srcp,
                                func=mybir.ActivationFunctionType.Copy,
                                bias=0.0,
                                scale=1.0,
                            )
                        else:
                            nc.vector.tensor_copy(out=dst, in_=srcp)
                    nc.sync.dma_start(
                        out=ho_out[
                            :, m * SEQ + 2 * qp * QW : m * SEQ + 2 * (qp + 1) * QW
                        ],
                        in_=ho[
                            :, m * SEQ + 2 * qp * QW : m * SEQ + 2 * (qp + 1) * QW
                        ],
                    )

    nc.compile()
    return nc


def _device_partials(x, W1, trace=False):
    """Run the d-sharded partial GEMM; returns preact [256, 2048] f32 sum."""
    global _NC
    if _NC is None:
        _NC = _build_nc()

    xt16 = np.ascontiguousarray(x.T.astype(np.float16))  # [4096, 2048]
    wt16 = np.ascontiguousarray(W1.T.astype(np.float16))  # [4096, 256]

    in_maps = []
    for c in range(N_CORES):
        blk = np.empty((128, KT * CHUNK), dtype=np.float16)
        for k in range(KT):
            d0 = c * KSLICE + k * 128
            blk[:, k * CHUNK : k * CHUNK + HID] = wt16[d0 : d0 + 128]
            blk[:, k * CHUNK + HID : (k + 1) * CHUNK] = xt16[d0 : d0 + 128]
        in_maps.append({"data": blk})

    res = run_bass_kernel_spmd(
        _NC, in_maps, core_ids=list(range(N_CORES)), trace=trace
    )
    acc = np.zeros((HID, SEQ), dtype=np.float32)
    for c in range(N_CORES):
        ho = res.results[c]["ho"]  # [128, MH*SEQ] f16
        for m in range(MH):
            acc[m * 128 : (m + 1) * 128] += ho[:, m * SEQ : (m + 1) * SEQ]
    return acc, res


def _tail(s, x, b2, R1, Rb1, R2, Rb2):
    """Host fp32 tail: softmax -> pool -> normalize -> router MLP -> top-k."""
    s = (s + np.float32(b2.reshape(-1)[0])).astype(np.float32)
    m = s.max()
    e = np.exp(s - m)
    attn = (e / e.sum()).astype(np.float32)
    pooled = (attn @ x).astype(np.float32)  # [4096]
    nrm = np.float32(np.sqrt(np.float64((pooled.astype(np.float64) ** 2).sum())))
    pooled = pooled / max(nrm, np.float32(1e-12)) * np.float32(math.sqrt(DIM))
    pre = (pooled @ R1.T + Rb1).astype(np.float32)
    erf = np.array(
        [math.erf(float(v) / math.sqrt(2.0)) for v in pre], dtype=np.float32
    )
    r = np.float32(0.5) * pre * (np.float32(1.0) + erf)
    logits = (r @ R2.T + Rb2).astype(np.float32)
    lt = logits / np.float32(TEMPERATURE)
    em = np.exp(lt - lt.max())
    probs = (em / em.sum()).astype(np.float32)
    idx = np.argsort(-probs, kind="stable")[:TOP_ROUTER].astype(np.int32)
    w = probs[idx]
    w = (w / w.sum()).astype(np.float32)
    return idx, w, probs


def _run(inputs, trace=False):
    x = np.asarray(inputs["text_features"], dtype=np.float32)[0]
    W1 = np.asarray(inputs["W1"], dtype=np.float32)
    b1 = np.asarray(inputs["b1"], dtype=np.float32)
    W2 = np.asarray(inputs["W2"], dtype=np.float32)
    b2 = np.asarray(inputs["b2"], dtype=np.float32)
    R1 = np.asarray(inputs["R1"], dtype=np.float32)
    Rb1 = np.asarray(inputs["Rb1"], dtype=np.float32)
    R2 = np.asarray(inputs["R2"], dtype=np.float32)
    Rb2 = np.asarray(inputs["Rb2"], dtype=np.float32)

    preact, res = _device_partials(x, W1, trace=trace)
    h = np.maximum(preact + b1[:, None], 0.0).astype(np.float32)  # [256, 2048]
    s = (W2.reshape(-1).astype(np.float32) @ h).astype(np.float32)  # [2048]
    out = _tail(s, x, b2, R1, Rb1, R2, Rb2)
    return out, res


def kernel(**inputs):
    out, _ = _run(inputs, trace=False)
    return out


# revision 15
# speedup vs baseline: 1.3581x; 1.0527x over previous
"""Trainium2 Bass kernel for nn_AttentionLayerRouter.

Reference semantics: attention-pool over text_features, then a router MLP +
top-k — but the returned outputs depend ONLY on batch sample 0
(`top_indices[0], top_weights[0], layer_probs[0]`), so samples 1..15 are dead
computation and are skipped entirely.

Device work (the compute-heavy 99%): the pre-activation GEMM for sample 0,
preact = x @ W1.T with x = text_features[0] ([2048, 4096]), sharded across
the 8 cores along the CONTRACTION dim (512 of 4096 per core). Each core
computes a [256, 2048] partial in fp16-in/fp32-accumulate and returns it as
fp16. Sharding the contraction keeps per-core DMA at 2.3MB and lets every
matmul run with a 512-wide moving operand (PSUM-bank-sized) while rotating
across all 8 PSUM banks for back-to-back issue.

Host tail (~1% of FLOPs): sum the 8 partials, bias+relu, scores, softmax
over 2048, attention-weighted pooling, l2-normalize, router MLP on a single
[4096] vector, top-8 of 24. Validated against the reference: fp16 operands
+ fp16 partials perturb the final logits by ~4e-7 vs a 1.5e-5 minimum
top-k gap.
"""

import math

import numpy as np

import concourse.mybir as mybir
import concourse.tile as tile
from concourse import bacc
from concourse.bass_utils import run_bass_kernel_spmd

N_CORES = 8
SEQ = 2048
DIM = 4096
HID = 256
NUM_LAYERS = 24
TOP_ROUTER = 8
TEMPERATURE = 2.0

KSLICE = DIM // N_CORES  # 512 contraction elements per core
KT = KSLICE // 128  # 4 k-tiles per core
NQ = 4  # seq quarters (moving-operand N = 512 = one PSUM bank)
QW = SEQ // NQ  # 512
MH = HID // 128  # 2 hid tiles
CHUNK = HID + SEQ  # packed columns per k-tile: [w1t_k | xt_k]

_NC = None


def _build_nc():
    """Per-core program: partial[m*128+p, l] = sum_d W1T[d, .] x[l, d]."""
    nc = bacc.Bacc(
        "TRN2", target_bir_lowering=False, debug=False, num_devices=N_CORES
    )
    f16, f32 = mybir.dt.float16, mybir.dt.float32

    # One packed input: per k-tile, 256 cols of W1.T then 2048 cols of x.T
    # (both laid out partition-major on the host so every DMA is direct-2D).
    data = nc.dram_tensor("data", [128, KT * CHUNK], f16, kind="ExternalInput")
    ho_out = nc.dram_tensor("ho", [128, MH * SEQ], f16, kind="ExternalOutput")

    with tile.TileContext(nc) as tc:
        with (
            tc.tile_pool(name="sb", bufs=1) as sb,
            tc.tile_pool(name="psum", bufs=1, space="PSUM") as psum,
        ):
            # Warm the ACT lookup table early so the first real ACT copy
            # doesn't pay the ~1.3us table load.
            warm = sb.tile([128, 1], f32)
            nc.vector.memset(warm[:, :], 0.0)
            warm2 = sb.tile([128, 1], f16)
            nc.scalar.activation(
                out=warm2[:, :],
                in_=warm[:, :],
                func=mybir.ActivationFunctionType.Copy,
                bias=0.0,
                scale=1.0,
            )
            # Warm-up matmuls on zeros while the first chunk is in flight:
            # the PE's HAM clock-gate needs ~3.4us of sustained activity to
            # reach full rate, so burn the DMA-wait on throwaway matmuls and
            # run the real chain entirely at 2.4GHz.
            wz = sb.tile([128, 640], f16)
            nc.vector.memset(wz[:, :], 0.0)

            chunks = []
            for k in range(KT):
                ch = sb.tile([128, CHUNK], f16, tag=f"ch{k}", name=f"ch{k}")
                nc.sync.dma_start(
                    out=ch[:, :], in_=data[:, k * CHUNK : (k + 1) * CHUNK]
                )
                chunks.append(ch)

            hp = [
                psum.tile([128, SEQ], f32, tag=f"hp{m}", name=f"hp{m}")
                for m in range(MH)
            ]
            for _ in range(11):
                nc.tensor.matmul(
                    hp[0][:, 0:QW],
                    wz[:, 0:128],
                    wz[:, 128:640],
                    start=True,
                    stop=True,
                    skip_group_check=True,
                )
            # k outer so matmuls track chunk arrivals; both hid-tiles
            # interleave per k (banks rotate across all 8 PSUM banks).
            ho = sb.tile([128, MH * SEQ], f16)
            for k in range(KT):
                ch = chunks[k]
                if k == KT - 1:
                    # Final pass: finish the DVE-evacuated banks (q0/q1)
                    # first so both evacuation engines start ASAP.
                    order = [(m, q) for q in range(NQ) for m in range(MH)]
                else:
                    order = [(m, q) for m in range(MH) for q in range(NQ)]
                for m, q in order:
                    nc.tensor.matmul(
                        hp[m][:, q * QW : (q + 1) * QW],
                        ch[:, m * 128 : (m + 1) * 128],
                        ch[:, HID + q * QW : HID + (q + 1) * QW],
                        start=(k == 0),
                        stop=(k == KT - 1),
                    )
            # Evacuate PSUM as f16 with DVE and ACT in parallel (q0/q1 on
            # DVE, q2/q3 on ACT); outputs stream per (m, q-pair) so the last
            # transfer is small.
            for m in range(MH):
                for qp in range(NQ // 2):
                    for q in (2 * qp, 2 * qp + 1):
                        dst = ho[:, m * SEQ + q * QW : m * SEQ + (q + 1) * QW]
                        srcp = hp[m][:, q * QW : (q + 1) * QW]
                        if q >= 2:
                            nc.scalar.activation(
                                out=dst,
                                in_=srcp,
                                func=mybir.ActivationFunctionType.Copy,
                                bias=0.0,
                                scale=1.0,
                            )
                        else:
                            nc.vector.tensor_copy(out=dst, in_=srcp)
                    nc.sync.dma_start(
                        out=ho_out[
                            :, m * SEQ + 2 * qp * QW : m * SEQ + 2 * (qp + 1) * QW
                        ],
                        in_=ho[
                            :, m * SEQ + 2 * qp * QW : m * SEQ + 2 * (qp + 1) * QW
                        ],
                    )

    nc.compile()
    return nc


def _device_partials(x, W1, trace=False):
    """Run the d-sharded partial GEMM; returns preact [256, 2048] f32 sum."""
    global _NC
    if _NC is None:
        _NC = _build_nc()

    xt16 = np.ascontiguousarray(x.T.astype(np.float16))  # [4096, 2048]
    wt16 = np.ascontiguousarray(W1.T.astype(np.float16))  # [4096, 256]

    in_maps = []
    for c in range(N_CORES):
        blk = np.empty((128, KT * CHUNK), dtype=np.float16)
        for k in range(KT):
            d0 = c * KSLICE + k * 128
            blk[:, k * CHUNK : k * CHUNK + HID] = wt16[d0 : d0 + 128]
            blk[:, k * CHUNK + HID : (k + 1) * CHUNK] = xt16[d0 : d0 + 128]
        in_maps.append({"data": blk})

    res = run_bass_kernel_spmd(
        _NC, in_maps, core_ids=list(range(N_CORES)), trace=trace
    )
    acc = np.zeros((HID, SEQ), dtype=np.float32)
    for c in range(N_CORES):
        ho = res.results[c]["ho"]  # [128, MH*SEQ] f16
        for m in range(MH):
            acc[m * 128 : (m + 1) * 128] += ho[:, m * SEQ : (m + 1) * SEQ]
    return acc, res


def _tail(s, x, b2, R1, Rb1, R2, Rb2):
    """Host fp32 tail: softmax -> pool -> normalize -> router MLP -> top-k."""
    s = (s + np.float32(b2.reshape(-1)[0])).astype(np.float32)
    m = s.max()
    e = np.exp(s - m)
    attn = (e / e.sum()).astype(np.float32)
    pooled = (attn @ x).astype(np.float32)  # [4096]
    nrm = np.float32(np.sqrt(np.float64((pooled.astype(np.float64) ** 2).sum())))
    pooled = pooled / max(nrm, np.float32(1e-12)) * np.float32(math.sqrt(DIM))
    pre = (pooled @ R1.T + Rb1).astype(np.float32)
    erf = np.array(
        [math.erf(float(v) / math.sqrt(2.0)) for v in pre], dtype=np.float32
    )
    r = np.float32(0.5) * pre * (np.float32(1.0) + erf)
    logits = (r @ R2.T + Rb2).astype(np.float32)
    lt = logits / np.float32(TEMPERATURE)
    em = np.exp(lt - lt.max())
    probs = (em / em.sum()).astype(np.float32)
    idx = np.argsort(-probs, kind="stable")[:TOP_ROUTER].astype(np.int32)
    w = probs[idx]
    w = (w / w.sum()).astype(np.float32)
    return idx, w, probs


def _run(inputs, trace=False):
    x = np.asarray(inputs["text_features"], dtype=np.float32)[0]
    W1 = np.asarray(inputs["W1"], dtype=np.float32)
    b1 = np.asarray(inputs["b1"], dtype=np.float32)
    W2 = np.asarray(inputs["W2"], dtype=np.float32)
    b2 = np.asarray(inputs["b2"], dtype=np.float32)
    R1 = np.asarray(inputs["R1"], dtype=np.float32)
    Rb1 = np.asarray(inputs["Rb1"], dtype=np.float32)
    R2 = np.asarray(inputs["R2"], dtype=np.float32)
    Rb2 = np.asarray(inputs["Rb2"], dtype=np.float32)

    preact, res = _device_partials(x, W1, trace=trace)
    h = np.maximum(preact + b1[:, None], 0.0).astype(np.float32)  # [256, 2048]
    s = (W2.reshape(-1).astype(np.float32) @ h).astype(np.float32)  # [2048]
    out = _tail(s, x, b2, R1, Rb1, R2, Rb2)
    return out, res


def kernel(**inputs):
    out, _ = _run(inputs, trace=False)
    return out


# revision 16
# speedup vs baseline: 1.5079x; 1.1103x over previous
"""Trainium2 Bass kernel for nn_AttentionLayerRouter.

Reference semantics: attention-pool over text_features, then a router MLP +
top-k — but the returned outputs depend ONLY on batch sample 0
(`top_indices[0], top_weights[0], layer_probs[0]`), so samples 1..15 are dead
computation and are skipped entirely.

Device work (the compute-heavy 99%): the pre-activation GEMM for sample 0,
preact = x @ W1.T with x = text_features[0] ([2048, 4096]), sharded across
the 8 cores along the CONTRACTION dim (512 of 4096 per core). Each core
computes a [256, 2048] partial in fp16-in/fp32-accumulate and returns it as
fp16. Sharding the contraction keeps per-core DMA at 2.3MB and lets every
matmul run with a 512-wide moving operand (PSUM-bank-sized) while rotating
across all 8 PSUM banks for back-to-back issue.

Host tail (~1% of FLOPs): sum the 8 partials, bias+relu, scores, softmax
over 2048, attention-weighted pooling, l2-normalize, router MLP on a single
[4096] vector, top-8 of 24. Validated against the reference: fp16 operands
+ fp16 partials perturb the final logits by ~4e-7 vs a 1.5e-5 minimum
top-k gap.
"""

import math

import numpy as np

import concourse.mybir as mybir
import concourse.tile as tile
from concourse import bacc
from concourse.bass_utils import run_bass_kernel_spmd

N_CORES = 8
SEQ = 2048
DIM = 4096
HID = 256
NUM_LAYERS = 24
TOP_ROUTER = 8
TEMPERATURE = 2.0

KSLICE = DIM // N_CORES  # 512 contraction elements per core
KT = KSLICE // 128  # 4 k-tiles per core
NQ = 4  # seq quarters (moving-operand N = 512 = one PSUM bank)
QW = SEQ // NQ  # 512
MH = HID // 128  # 2 hid tiles
CHUNK = HID + SEQ  # packed columns per k-tile: [w1t_k | xt_k]

_NC = None


def _build_nc():
    """Per-core program: partial[m*128+p, l] = sum_d W1T[d, .] x[l, d]."""
    nc = bacc.Bacc(
        "TRN2", target_bir_lowering=False, debug=False, num_devices=N_CORES
    )
    f16, f32 = mybir.dt.float16, mybir.dt.float32

    # One packed input: per k-tile, 256 cols of W1.T then 2048 cols of x.T
    # (both laid out partition-major on the host so every DMA is direct-2D).
    data = nc.dram_tensor("data", [128, KT * CHUNK], f16, kind="ExternalInput")
    ho_out = nc.dram_tensor("ho", [128, MH * SEQ], f16, kind="ExternalOutput")

    with tile.TileContext(nc) as tc:
        with (
            tc.tile_pool(name="sb", bufs=1) as sb,
            tc.tile_pool(name="psum", bufs=1, space="PSUM") as psum,
        ):
            # Warm the ACT lookup table early so the first real ACT copy
            # doesn't pay the ~1.3us table load.
            warm = sb.tile([128, 1], f32)
            nc.vector.memset(warm[:, :], 0.0)
            warm2 = sb.tile([128, 1], f16)
            nc.scalar.activation(
                out=warm2[:, :],
                in_=warm[:, :],
                func=mybir.ActivationFunctionType.Copy,
                bias=0.0,
                scale=1.0,
            )
            # Warm-up matmuls on zeros while the first chunk is in flight:
            # the PE's HAM clock-gate needs ~3.4us of sustained activity to
            # reach full rate, so burn the DMA-wait on throwaway matmuls and
            # run the real chain entirely at 2.4GHz.
            wz = sb.tile([128, 640], f16)
            nc.vector.memset(wz[:, :], 0.0)

            chunks = []
            for k in range(KT):
                ch = sb.tile([128, CHUNK], f16, tag=f"ch{k}", name=f"ch{k}")
                nc.sync.dma_start(
                    out=ch[:, :], in_=data[:, k * CHUNK : (k + 1) * CHUNK]
                )
                chunks.append(ch)

            # One PSUM tile per bank so evacuation deps are per-bank exact.
            hp = {}
            for m in range(MH):
                for q in range(NQ):
                    hp[(m, q)] = psum.tile(
                        [128, QW], f32, tag=f"hp{m}_{q}", name=f"hp{m}_{q}"
                    )
            for _ in range(11):
                nc.tensor.matmul(
                    hp[(0, 0)][:, :],
                    wz[:, 0:128],
                    wz[:, 128:640],
                    start=True,
                    stop=True,
                    skip_group_check=True,
                )
            # k outer so matmuls track chunk arrivals; both hid-tiles
            # interleave per k (banks rotate across all 8 PSUM banks).
            ho = sb.tile([128, MH * SEQ], f16)
            for k in range(KT):
                ch = chunks[k]
                if k == KT - 1:
                    # Final pass: banks complete in evacuation order.
                    order = [(m, q) for q in range(NQ) for m in range(MH)]
                else:
                    order = [(m, q) for m in range(MH) for q in range(NQ)]
                for m, q in order:
                    nc.tensor.matmul(
                        hp[(m, q)][:, :],
                        ch[:, m * 128 : (m + 1) * 128],
                        ch[:, HID + q * QW : HID + (q + 1) * QW],
                        start=(k == 0),
                        stop=(k == KT - 1),
                    )
            # Evacuate PSUM as f16 with DVE (q0/q1 banks) and ACT (q2/q3)
            # in parallel, interleaved in bank-completion order; each
            # engine's pair of banks per m streams out as its own DMA.
            for q0, q1 in ((0, 1), (2, 3)):
                for m in range(MH):
                    for q in (q0, q1):
                        dst = ho[:, m * SEQ + q * QW : m * SEQ + (q + 1) * QW]
                        srcp = hp[(m, q)][:, :]
                        if q >= 2:
                            nc.scalar.activation(
                                out=dst,
                                in_=srcp,
                                func=mybir.ActivationFunctionType.Copy,
                                bias=0.0,
                                scale=1.0,
                            )
                        else:
                            nc.vector.tensor_copy(out=dst, in_=srcp)
                    nc.sync.dma_start(
                        out=ho_out[
                            :, m * SEQ + q0 * QW : m * SEQ + (q1 + 1) * QW
                        ],
                        in_=ho[:, m * SEQ + q0 * QW : m * SEQ + (q1 + 1) * QW],
                    )

    nc.compile()
    return nc


def _device_partials(x, W1, trace=False):
    """Run the d-sharded partial GEMM; returns preact [256, 2048] f32 sum."""
    global _NC
    if _NC is None:
        _NC = _build_nc()

    xt16 = np.ascontiguousarray(x.T.astype(np.float16))  # [4096, 2048]
    wt16 = np.ascontiguousarray(W1.T.astype(np.float16))  # [4096, 256]

    in_maps = []
    for c in range(N_CORES):
        blk = np.empty((128, KT * CHUNK), dtype=np.float16)
        for k in range(KT):
            d0 = c * KSLICE + k * 128
            blk[:, k * CHUNK : k * CHUNK + HID] = wt16[d0 : d0 + 128]
            blk[:, k * CHUNK + HID : (k + 1) * CHUNK] = xt16[d0 : d0 + 128]
        in_maps.append({"data": blk})

    res = run_bass_kernel_spmd(
        _NC, in_maps, core_ids=list(range(N_CORES)), trace=trace
    )
    acc = np.zeros((HID, SEQ), dtype=np.float32)
    for c in range(N_CORES):
        ho = res.results[c]["ho"]  # [128, MH*SEQ] f16
        for m in range(MH):
            acc[m * 128 : (m + 1) * 128] += ho[:, m * SEQ : (m + 1) * SEQ]
    return acc, res


def _tail(s, x, b2, R1, Rb1, R2, Rb2):
    """Host fp32 tail: softmax -> pool -> normalize -> router MLP -> top-k."""
    s = (s + np.float32(b2.reshape(-1)[0])).astype(np.float32)
    m = s.max()
    e = np.exp(s - m)
    attn = (e / e.sum()).astype(np.float32)
    pooled = (attn @ x).astype(np.float32)  # [4096]
    nrm = np.float32(np.sqrt(np.float64((pooled.astype(np.float64) ** 2).sum())))
    pooled = pooled / max(nrm, np.float32(1e-12)) * np.float32(math.sqrt(DIM))
    pre = (pooled @ R1.T + Rb1).astype(np.float32)
    erf = np.array(
        [math.erf(float(v) / math.sqrt(2.0)) for v in pre], dtype=np.float32
    )
    r = np.float32(0.5) * pre * (np.float32(1.0) + erf)
    logits = (r @ R2.T + Rb2).astype(np.float32)
    lt = logits / np.float32(TEMPERATURE)
    em = np.exp(lt - lt.max())
    probs = (em / em.sum()).astype(np.float32)
    idx = np.argsort(-probs, kind="stable")[:TOP_ROUTER].astype(np.int32)
    w = probs[idx]
    w = (w / w.sum()).astype(np.float32)
    return idx, w, probs


def _run(inputs, trace=False):
    x = np.asarray(inputs["text_features"], dtype=np.float32)[0]
    W1 = np.asarray(inputs["W1"], dtype=np.float32)
    b1 = np.asarray(inputs["b1"], dtype=np.float32)
    W2 = np.asarray(inputs["W2"], dtype=np.float32)
    b2 = np.asarray(inputs["b2"], dtype=np.float32)
    R1 = np.asarray(inputs["R1"], dtype=np.float32)
    Rb1 = np.asarray(inputs["Rb1"], dtype=np.float32)
    R2 = np.asarray(inputs["R2"], dtype=np.float32)
    Rb2 = np.asarray(inputs["Rb2"], dtype=np.float32)

    preact, res = _device_partials(x, W1, trace=trace)
    h = np.maximum(preact + b1[:, None], 0.0).astype(np.float32)  # [256, 2048]
    s = (W2.reshape(-1).astype(np.float32) @ h).astype(np.float32)  # [2048]
    out = _tail(s, x, b2, R1, Rb1, R2, Rb2)
    return out, res


def kernel(**inputs):
    out, _ = _run(inputs, trace=False)
    return out
